# revision 1
# baseline (speedup 1.0000x reference)
"""Trainium2 Bass kernel for CaMoE (LN + top-2 MoE with relu^2 FFN).

Strategy: expert-parallel over 8 NeuronCores. Core e receives only the
tokens routed to expert e (gather indices computed host-side as part of
sharding), plus W1[e]/W2[e] in bf16, pre-swizzled into per-tile lhsT
layout. On device: LayerNorm stats via ones-matmul in replicated-lane
form (stats vectors come out already broadcast across partitions),
xn = (x - mu) * rstd * sqrt(coef) in bf16, hidden = relu(xn @ W1)^2
with fp32 PSUM accumulation, y = hidden @ W2, written back
feature-major. Host scatter-adds the 8 partial outputs into x (the
residual) — pure unsharding, no collectives needed.

Self-contained: hardcodes shapes B=4, T=2048, C=1024, E=8, H=4096.
"""

import os
import sys

for _p in ("/opt/trn_rl_repo", "/root/.axon_site/_ro/trn_rl_repo"):
    if os.path.isdir(_p) and _p not in sys.path:
        sys.path.insert(0, _p)

from contextlib import ExitStack

import ml_dtypes
import numpy as np

import concourse.bass as bass
import concourse.tile as tile
from concourse import bacc, mybir
from concourse.bass_utils import run_bass_kernel_spmd

N_CORES = 8
C = 1024
H = 4096
NB = 512          # token block (matmul moving free dim)
NC_T = C // 128   # 8 c-tiles
NH_T = H // 128   # 32 h-tiles
EPS = 1e-5

F32 = mybir.dt.float32
BF16 = mybir.dt.bfloat16
AF = mybir.ActivationFunctionType
OP = mybir.AluOpType


def _build_kernel(NT: int, has_beta: bool):
    """Build the per-core SPMD program for NT padded tokens."""
    blocks = []
    t0 = 0
    while t0 < NT:
        tn = min(NB, NT - t0)
        blocks.append((t0, tn))
        t0 += tn
    nblk = len(blocks)
    nc = bacc.Bacc("TRN2", target_bir_lowering=False, debug=False, num_devices=1)

    xgt_d = nc.dram_tensor("xgt", [C, NT], F32, kind="ExternalInput").ap()
    # weights pre-swizzled on host into per-tile lhsT layout:
    #   w1[h][p, c*128+j] = (gamma*W1)[c*128+p, h*128+j]
    #   w2[c][p, h*128+j] = W2[h*128+p, c*128+j]
    w1_d = nc.dram_tensor("w1", [NH_T, 128, C], BF16, kind="ExternalInput").ap()
    w2_d = nc.dram_tensor("w2", [NC_T, 128, H], BF16, kind="ExternalInput").ap()
    cg_d = nc.dram_tensor("cg", [1, NT], F32, kind="ExternalInput").ap()
    if has_beta:
        bias1_d = nc.dram_tensor("bias1", [128, NH_T], F32, kind="ExternalInput").ap()
    ygt_d = nc.dram_tensor("ygt", [C, NT], F32, kind="ExternalOutput").ap()

    with tile.TileContext(nc) as tc, ExitStack() as ctx:
        sb = ctx.enter_context(tc.tile_pool(name="sb", bufs=1))
        ps = ctx.enter_context(tc.tile_pool(name="ps", bufs=1, space="PSUM"))

        # ---- constants ----
        ones_k = sb.tile([128, 128], BF16, tag="ones_k", bufs=1)
        nc.vector.memset(ones_k, 1.0)
        eps_t = sb.tile([128, 1], F32, tag="eps", bufs=1)
        nc.vector.memset(eps_t, EPS)
        if has_beta:
            b1sb = sb.tile([128, NH_T], F32, tag="b1", bufs=1)
            nc.sync.dma_start(b1sb, bias1_d)

        def stats_phase(blk):
            """LN stats for block blk, replicated-lane form.

            Returns [128,tn] scale/shift (already broadcast across
            partitions) plus the raw x tiles (kept for normalize)."""
            t0, tn = blocks[blk]
            tsl = bass.ds(t0, tn)
            sum_ps = ps.tile([128, tn], F32, tag="stat", bufs=3, name=f"sum{blk}")
            sq_ps = ps.tile([128, tn], F32, tag="stat", bufs=3, name=f"sq{blk}")
            xs = []
            for c in range(NC_T):
                xt = sb.tile([128, tn], F32, tag="xs", bufs=14, name=f"xa{blk}_{c}", padded_shape=[128, NB])
                nc.sync.dma_start(xt, xgt_d[c * 128:(c + 1) * 128, tsl])
                xb = sb.tile([128, tn], BF16, tag="xb16", bufs=3, name=f"xb16{blk}_{c}", padded_shape=[128, NB])
                nc.vector.tensor_copy(xb, xt)
                xsq = sb.tile([128, tn], BF16, tag="xsq", bufs=3, name=f"xsq{blk}_{c}", padded_shape=[128, NB])
                nc.scalar.activation(xsq, xt, AF.Square)
                nc.tensor.matmul(sum_ps, ones_k, xb,
                                 start=(c == 0), stop=(c == NC_T - 1))
                nc.tensor.matmul(sq_ps, ones_k, xsq,
                                 start=(c == 0), stop=(c == NC_T - 1))
                xs.append(xt)
            vmu = sb.tile([128, tn], F32, tag="vec", bufs=3, name=f"vmu{blk}", padded_shape=[128, NB])
            nc.vector.tensor_scalar_mul(vmu, sum_ps, 1.0 / C)
            # var = sq/C - mu^2
            vvar = sb.tile([128, tn], F32, tag="vec", bufs=3, name=f"vvar{blk}", padded_shape=[128, NB])
            nc.vector.scalar_tensor_tensor(vvar, vmu, -1.0, vmu, OP.mult, OP.mult)
            nc.vector.scalar_tensor_tensor(vvar, sq_ps, 1.0 / C, vvar, OP.mult, OP.add)
            vstd = sb.tile([128, tn], F32, tag="vec", bufs=3, name=f"vstd{blk}", padded_shape=[128, NB])
            nc.scalar.activation(vstd, vvar, AF.Sqrt, bias=eps_t)
            vrstd = sb.tile([128, tn], F32, tag="vec", bufs=3, name=f"vrstd{blk}", padded_shape=[128, NB])
            nc.vector.reciprocal_approx_fast(out=vrstd, in_=vstd)
            vcg = sb.tile([128, tn], F32, tag="bc", bufs=8, name=f"vcg{blk}", padded_shape=[128, NB])
            nc.sync.dma_start(vcg, cg_d[0:1, tsl].to_broadcast([128, tn]))
            if has_beta:
                vs = vrstd          # coef applied on the output instead
            else:
                vs = sb.tile([128, tn], F32, tag="bc", bufs=8, name=f"vs{blk}", padded_shape=[128, NB])
                nc.vector.tensor_mul(vs, vrstd, vcg)
            vb = sb.tile([128, tn], F32, tag="bc", bufs=8, name=f"vb{blk}", padded_shape=[128, NB])
            nc.vector.scalar_tensor_tensor(vb, vmu, -1.0, vs, OP.mult, OP.mult)
            return vs, vb, vcg, xs

        def normalize_phase(blk, vs, vb, xs):
            t0, tn = blocks[blk]
            xn = []
            for c in range(NC_T):
                xt = xs[c]
                nc.vector.tensor_mul(xt, xt, vs)
                xnc = sb.tile([128, tn], BF16, tag="xn", bufs=20, name=f"xn{blk}_{c}", padded_shape=[128, NB])
                nc.vector.tensor_add(xnc, xt, vb)
                xn.append(xnc)
            return xn

        def mm1_phase(blk, xn, mid_hook=None):
            t0, tn = blocks[blk]
            hid = []
            for h in range(NH_T):
                if h == 16 and mid_hook is not None:
                    mid_hook()
                w1t = sb.tile([128, C], BF16, tag="w1s", bufs=8, name=f"w1t{blk}_{h}")
                nc.scalar.dma_start(w1t, w1_d[h])
                pa = ps.tile([128, tn], F32, tag="mm", bufs=4, name=f"pa{blk}_{h}")
                for c in range(NC_T):
                    nc.tensor.matmul(pa, w1t[:, c * 128:(c + 1) * 128], xn[c],
                                     start=(c == 0), stop=(c == NC_T - 1))
                if has_beta:
                    nc.vector.tensor_scalar_add(pa, pa, b1sb[:, h:h + 1])
                # relu(x)^2 == max(x,0)*x; DVE may read only one PSUM operand
                rt = sb.tile([128, tn], BF16, tag="rt", bufs=3, name=f"r{blk}_{h}", padded_shape=[128, NB])
                nc.vector.tensor_scalar_max(rt, pa, 0.0)
                ht = sb.tile([128, tn], BF16, tag="hid", bufs=44, name=f"h{blk}_{h}", padded_shape=[128, NB])
                nc.vector.tensor_mul(ht, rt, pa)
                hid.append(ht)
            return hid

        def mm2_phase(blk, hid, vcf):
            t0, tn = blocks[blk]
            tsl = bass.ds(t0, tn)
            for c in range(NC_T):
                w2t = sb.tile([128, H], BF16, tag="w2s", bufs=4, name=f"w2t{blk}_{c}")
                nc.scalar.dma_start(w2t, w2_d[c])
                pb = ps.tile([128, tn], F32, tag="mm", bufs=4, name=f"pb{blk}_{c}")
                for h in range(NH_T):
                    nc.tensor.matmul(pb, w2t[:, h * 128:(h + 1) * 128], hid[h],
                                     start=(h == 0), stop=(h == NH_T - 1))
                ot = sb.tile([128, tn], F32, tag="out", bufs=4, name=f"o{blk}_{c}", padded_shape=[128, NB])
                if has_beta:
                    nc.vector.tensor_mul(ot, pb, vcf)
                else:
                    nc.vector.tensor_copy(ot, pb)
                nc.sync.dma_start(ygt_d[c * 128:(c + 1) * 128, tsl], ot)

        # Software pipeline: stats/normalize of blk+1 are emitted so the PE
        # runs them inside blk's mm1/mm2 stream with no gaps.
        vs0, vb0, vcf, xs0 = stats_phase(0)
        xn = normalize_phase(0, vs0, vb0, xs0)
        nxt = {}
        for blk in range(nblk):
            def mid_hook(b=blk):
                nxt.update(zip(("vs", "vb", "vcf", "xs"), stats_phase(b + 1)))
            hid = mm1_phase(blk, xn, mid_hook if blk + 1 < nblk else None)
            if blk + 1 < nblk:
                xn = normalize_phase(blk + 1, nxt["vs"], nxt["vb"], nxt["xs"])
            mm2_phase(blk, hid, vcf)
            if blk + 1 < nblk:
                vcf = nxt["vcf"]

    nc.compile()
    return nc


_KERNEL_CACHE = {}


def _get_kernel(NT: int, has_beta: bool):
    key = (NT, has_beta)
    if key not in _KERNEL_CACHE:
        _KERNEL_CACHE[key] = _build_kernel(NT, has_beta)
    return _KERNEL_CACHE[key]


def kernel(x, weights, gamma, beta, W1, W2, winners):
    x = np.asarray(x, dtype=np.float32)
    weights = np.asarray(weights, dtype=np.float32)
    gamma = np.asarray(gamma, dtype=np.float32)
    beta = np.asarray(beta, dtype=np.float32)
    W1 = np.asarray(W1, dtype=np.float32)
    W2 = np.asarray(W2, dtype=np.float32)
    winners = np.asarray(winners)

    B, T, C_ = x.shape
    E = W1.shape[0]
    assert C_ == C and E == N_CORES and W1.shape[2] == H

    x_flat = x.reshape(-1, C)
    win = winners.reshape(-1, 2)
    wts = weights.reshape(-1, 2)

    has_beta = bool(np.any(beta != 0.0))

    # ---- host-side routing (sharding prep) ----
    idxs, coefs = [], []
    for e in range(E):
        m = win == e
        tok = np.nonzero(m.any(axis=1))[0]
        cf = (wts * m).sum(axis=1)[tok]
        idxs.append(tok)
        coefs.append(cf.astype(np.float32))
    NT = int(np.ceil(max(len(t) for t in idxs) / 8) * 8)

    in_maps = []
    for e in range(E):
        tok, cf = idxs[e], coefs[e]
        n = len(tok)
        xg = np.zeros((NT, C), np.float32)
        xg[:n] = x_flat[tok]
        cg = np.zeros((1, NT), np.float32)
        # no beta: fold sqrt(coef) into the LN scale (relu^2 is 2-homogeneous
        # and W2 linear, so scaling xn by sqrt(c) scales the output by c).
        cg[0, :n] = cf if has_beta else np.sqrt(cf)
        w1g = (W1[e] * gamma[:, None]).astype(ml_dtypes.bfloat16)
        w1r = np.ascontiguousarray(
            w1g.reshape(NC_T, 128, NH_T, 128).transpose(2, 1, 0, 3)
        ).reshape(NH_T, 128, C)
        w2r = np.ascontiguousarray(
            W2[e].astype(ml_dtypes.bfloat16)
            .reshape(NH_T, 128, NC_T, 128).transpose(2, 1, 0, 3)
        ).reshape(NC_T, 128, H)
        m = {
            "xgt": np.ascontiguousarray(xg.T),
            "w1": w1r,
            "w2": w2r,
            "cg": cg,
        }
        if has_beta:
            b1 = (beta @ W1[e]).astype(np.float32)          # [H]
            m["bias1"] = np.ascontiguousarray(b1.reshape(NH_T, 128).T)
        in_maps.append(m)

    nc = _get_kernel(NT, has_beta)
    res = run_bass_kernel_spmd(nc, in_maps, list(range(N_CORES)))

    # ---- host-side unshard: scatter-add partial expert outputs ----
    out = x_flat.copy()
    for e in range(E):
        yg = res.results[e]["ygt"]                          # [C, NT]
        n = len(idxs[e])
        out[idxs[e]] += yg.T[:n]
    return out.reshape(B, T, C).astype(np.float32)



# revision 2
# speedup vs baseline: 1.0563x; 1.0563x over previous
"""Trainium2 Bass kernel for CaMoE (LN + top-2 MoE with relu^2 FFN).

Strategy: expert-parallel over 8 NeuronCores with coef-routed mixed
precision. Core e receives the tokens routed to expert e (gather
indices computed host-side as part of sharding), sorted by combine
coefficient ascending. The first NF8 tokens (lowest coef) run both
matmuls in fp8-e4m3 DoubleRow (2x PE throughput), the next NBF run
mm1 in bf16 / mm2 in fp8 DoubleRow, the rest run fully in bf16. The
combine coefficient bounds each pair's contribution to the output, so
quantization error from the fp8 classes stays ~coef-proportional and
small; measured absmax/scale ~1.5e-2 vs the 2e-2 gate.

On device: LayerNorm stats via ones-matmul in replicated-lane form,
xn = (x - mu) * rstd * sqrt(coef) (relu^2 is 2-homogeneous and W2
linear, so scaling xn by sqrt(c) scales the output by c), hidden =
relu(xn @ W1)^2 with fp32 PSUM accumulation, y = hidden @ W2, written
back feature-major. Host scatter-adds the 8 partial outputs into x
(the residual) - pure unsharding, no collectives needed.

Self-contained: hardcodes shapes B=4, T=2048, C=1024, E=8, H=4096.
"""

import os
import sys

for _p in ("/opt/trn_rl_repo", "/root/.axon_site/_ro/trn_rl_repo"):
    if os.path.isdir(_p) and _p not in sys.path:
        sys.path.insert(0, _p)

from contextlib import ExitStack

import ml_dtypes
import numpy as np

import concourse.bass as bass
import concourse.tile as tile
from concourse import bacc, mybir
from concourse.bass_utils import run_bass_kernel_spmd

N_CORES = 8
C = 1024
H = 4096
NB = 512          # token block (matmul moving free dim)
NC_T = C // 128   # 8 c-tiles
NH_T = H // 128   # 32 h-tiles
EPS = 1e-5

# mixed-precision class sizes (tokens, sorted by coef ascending)
NF8 = 768         # both matmuls fp8 DoubleRow
NBF = 512         # mm1 bf16, mm2 fp8 DoubleRow
# fp8 scale factors
S_X = 16.0        # xn pre-scale (fp8 class FF)
S_1 = 128.0       # W1 pre-scale (fp8)
S_H = 4.0         # hidden pre-scale (fp8)
S_2 = 256.0       # W2 pre-scale (fp8)

F32 = mybir.dt.float32
BF16 = mybir.dt.bfloat16
FP8 = mybir.dt.float8e4
DR = mybir.MatmulPerfMode.DoubleRow
AF = mybir.ActivationFunctionType
OP = mybir.AluOpType
NP_FP8 = mybir.dt.np(FP8)
NP_BF16 = mybir.dt.np(BF16)


def _block_list(NT, nf8, nbf):
    """[(t0, tn, cls)] covering [0, NT). FF blocks ordered small-first."""
    blocks = []

    def span(lo, hi, cls, small_first=False):
        chunks = []
        t = lo
        while t < hi:
            tn = min(NB, hi - t)
            chunks.append([t, tn, cls])
            t += tn
        if small_first and len(chunks) > 1:
            # move the remainder chunk to the front of the span
            sizes = sorted((ch[1] for ch in chunks))
            t = lo
            for ch, sz in zip(chunks, sizes):
                ch[0], ch[1] = t, sz
                t += sz
        blocks.extend(tuple(ch) for ch in chunks)

    b0 = min(nf8, NT)
    b1 = min(nf8 + nbf, NT)
    span(0, b0, "FF", small_first=True)
    span(b0, b1, "BF")
    span(b1, NT, "BB")
    return blocks


def _build_kernel(NT: int, has_beta: bool):
    """Build the per-core SPMD program for NT padded tokens."""
    nf8, nbf = (0, 0) if has_beta else (NF8, NBF)
    blocks = _block_list(NT, nf8, nbf)
    nblk = len(blocks)
    any_ff = any(b[2] == "FF" for b in blocks)
    any_f8mm2 = any(b[2] in ("FF", "BF") for b in blocks)
    any_bf16mm1 = any(b[2] in ("BF", "BB") for b in blocks)
    any_bb = any(b[2] == "BB" for b in blocks)

    nc = bacc.Bacc("TRN2", target_bir_lowering=False, debug=False, num_devices=1)

    xgt_d = nc.dram_tensor("xgt", [C, NT], BF16, kind="ExternalInput").ap()
    # weights pre-swizzled on host into per-tile lhsT layout:
    #   w1b[h][p, c, j] = (gamma*W1)[c*128+p, h*128+j]         (bf16)
    #   w1f[h][p, c, j] = (gamma*W1*S_1)[c*128+p, h*128+j]     (fp8)
    #   w2b[c][p, h*128+j] = W2[h*128+p, c*128+j]              (bf16)
    #   w2f[c][p, h, j] = (W2*S_2)[h*128+p, c*128+j]           (fp8)
    if any_bf16mm1:
        w1b_d = nc.dram_tensor("w1b", [NH_T, 128, C], BF16, kind="ExternalInput").ap()
    if any_ff:
        w1f_d = nc.dram_tensor("w1f", [NH_T, 128, NC_T, 128], FP8,
                               kind="ExternalInput").ap()
    if any_bb:
        w2b_d = nc.dram_tensor("w2b", [NC_T, 128, H], BF16, kind="ExternalInput").ap()
    if any_f8mm2:
        w2f_d = nc.dram_tensor("w2f", [NC_T, 128, NH_T, 128], FP8,
                               kind="ExternalInput").ap()
    cg_d = nc.dram_tensor("cg", [1, NT], F32, kind="ExternalInput").ap()
    if has_beta:
        bias1_d = nc.dram_tensor("bias1", [128, NH_T], F32, kind="ExternalInput").ap()
    ygt_d = nc.dram_tensor("ygt", [C, NT], F32, kind="ExternalOutput").ap()

    K_FF = S_H / (S_X * S_1) ** 2
    K_BF = S_H
    DESC = 1.0 / (S_H * S_2)

    with tile.TileContext(nc) as tc, ExitStack() as ctx:
        sb = ctx.enter_context(tc.tile_pool(name="sb", bufs=1))
        ps = ctx.enter_context(tc.tile_pool(name="ps", bufs=1, space="PSUM"))

        # ---- constants ----
        ones_k = sb.tile([128, 128], BF16, tag="ones_k", bufs=1)
        nc.vector.memset(ones_k, 1.0)
        eps_t = sb.tile([128, 1], F32, tag="eps", bufs=1)
        nc.vector.memset(eps_t, EPS)
        if has_beta:
            b1sb = sb.tile([128, NH_T], F32, tag="b1", bufs=1)
            nc.sync.dma_start(b1sb, bias1_d)

        def stats_phase(blk):
            """LN stats for block blk, replicated-lane form.

            Returns [128,tn] scale/shift (already broadcast across
            partitions) plus the raw x tiles (kept for normalize)."""
            t0, tn, cls = blocks[blk]
            tsl = bass.ds(t0, tn)
            sum_ps = ps.tile([128, tn], F32, tag="stat", bufs=3, name=f"sum{blk}")
            sq_ps = ps.tile([128, tn], F32, tag="stat", bufs=3, name=f"sq{blk}")
            xs = []
            for c in range(NC_T):
                xt = sb.tile([128, tn], BF16, tag="xs", bufs=12, name=f"xa{blk}_{c}", padded_shape=[128, NB])
                nc.sync.dma_start(xt, xgt_d[c * 128:(c + 1) * 128, tsl])
                xsq = sb.tile([128, tn], BF16, tag="xsq", bufs=3, name=f"xsq{blk}_{c}", padded_shape=[128, NB])
                nc.scalar.activation(xsq, xt, AF.Square)
                nc.tensor.matmul(sum_ps, ones_k, xt,
                                 start=(c == 0), stop=(c == NC_T - 1))
                nc.tensor.matmul(sq_ps, ones_k, xsq,
                                 start=(c == 0), stop=(c == NC_T - 1))
                xs.append(xt)
            vmu = sb.tile([128, tn], F32, tag="vec", bufs=3, name=f"vmu{blk}", padded_shape=[128, NB])
            nc.vector.tensor_scalar_mul(vmu, sum_ps, 1.0 / C)
            # var = sq/C - mu^2
            vvar = sb.tile([128, tn], F32, tag="vec", bufs=3, name=f"vvar{blk}", padded_shape=[128, NB])
            nc.vector.scalar_tensor_tensor(vvar, vmu, -1.0, vmu, OP.mult, OP.mult)
            nc.vector.scalar_tensor_tensor(vvar, sq_ps, 1.0 / C, vvar, OP.mult, OP.add)
            vstd = sb.tile([128, tn], F32, tag="vec", bufs=3, name=f"vstd{blk}", padded_shape=[128, NB])
            nc.scalar.activation(vstd, vvar, AF.Sqrt, bias=eps_t)
            vrstd = sb.tile([128, tn], F32, tag="vec", bufs=3, name=f"vrstd{blk}", padded_shape=[128, NB])
            nc.vector.reciprocal_approx_fast(out=vrstd, in_=vstd)
            vcg = sb.tile([128, tn], F32, tag="bc", bufs=6, name=f"vcg{blk}", padded_shape=[128, NB])
            nc.sync.dma_start(vcg, cg_d[0:1, tsl].to_broadcast([128, tn]))
            if has_beta:
                vs = vrstd          # coef applied on the output instead
            else:
                vs = sb.tile([128, tn], F32, tag="bc", bufs=6, name=f"vs{blk}", padded_shape=[128, NB])
                sxf = S_X if cls == "FF" else 1.0
                nc.vector.scalar_tensor_tensor(vs, vrstd, sxf, vcg, OP.mult, OP.mult)
            vb = sb.tile([128, tn], F32, tag="bc", bufs=6, name=f"vb{blk}", padded_shape=[128, NB])
            nc.vector.scalar_tensor_tensor(vb, vmu, -1.0, vs, OP.mult, OP.mult)
            return vs, vb, vcg, xs

        def normalize_phase(blk, vs, vb, xs):
            t0, tn, cls = blocks[blk]
            if cls == "FF":
                xn = sb.tile([128, NC_T, tn], FP8, tag="xnf", bufs=2,
                             name=f"xn{blk}", padded_shape=[128, NC_T, NB])
            else:
                xn = sb.tile([128, NC_T, tn], BF16, tag="xnb", bufs=2,
                             name=f"xn{blk}", padded_shape=[128, NC_T, NB])
            for c in range(NC_T):
                tmp = sb.tile([128, tn], F32, tag="tmp", bufs=3, name=f"tp{blk}_{c}", padded_shape=[128, NB])
                nc.vector.tensor_mul(tmp, xs[c], vs)
                nc.vector.tensor_add(xn[:, c, :], tmp, vb)
            return xn

        def mm1_phase(blk, xn, mid_hook=None):
            t0, tn, cls = blocks[blk]
            if cls == "BB":
                hid = sb.tile([128, NH_T, tn], BF16, tag="hidb", bufs=2,
                              name=f"hid{blk}", padded_shape=[128, NH_T, NB])
            else:
                hid = sb.tile([128, NH_T, tn], FP8, tag="hidf", bufs=2,
                              name=f"hid{blk}", padded_shape=[128, NH_T, NB])
            for h in range(NH_T):
                if h == 16 and mid_hook is not None:
                    mid_hook()
                pa = ps.tile([128, tn], F32, tag="mm", bufs=4, name=f"pa{blk}_{h}")
                if cls == "FF":
                    w1t = sb.tile([128, NC_T, 128], FP8, tag="w1f", bufs=4, name=f"w1t{blk}_{h}")
                    nc.scalar.dma_start(w1t, w1f_d[h])
                    for c in range(0, NC_T, 2):
                        nc.tensor.matmul(pa, w1t[:, c:c + 2, :], xn[:, c:c + 2, :],
                                         start=(c == 0), stop=(c == NC_T - 2),
                                         perf_mode=DR)
                else:
                    w1t = sb.tile([128, C], BF16, tag="w1s", bufs=4, name=f"w1t{blk}_{h}")
                    nc.scalar.dma_start(w1t, w1b_d[h])
                    for c in range(NC_T):
                        nc.tensor.matmul(pa, w1t[:, c * 128:(c + 1) * 128], xn[:, c, :],
                                         start=(c == 0), stop=(c == NC_T - 1))
                if has_beta:
                    nc.vector.tensor_scalar_add(pa, pa, b1sb[:, h:h + 1])
                # relu(x)^2 == max(x,0)*x; DVE may read only one PSUM operand
                rt = sb.tile([128, tn], BF16, tag="rt", bufs=3, name=f"r{blk}_{h}", padded_shape=[128, NB])
                nc.vector.tensor_scalar_max(rt, pa, 0.0)
                if cls == "BB":
                    nc.vector.tensor_mul(hid[:, h, :], rt, pa)
                else:
                    k = K_FF if cls == "FF" else K_BF
                    nc.vector.scalar_tensor_tensor(hid[:, h, :], pa, k, rt,
                                                   OP.mult, OP.mult)
            return hid

        def mm2_phase(blk, hid, vcf):
            t0, tn, cls = blocks[blk]
            tsl = bass.ds(t0, tn)
            for c in range(NC_T):
                pb = ps.tile([128, tn], F32, tag="mm", bufs=4, name=f"pb{blk}_{c}")
                if cls == "BB":
                    w2t = sb.tile([128, H], BF16, tag="w2s", bufs=2, name=f"w2t{blk}_{c}")
                    nc.scalar.dma_start(w2t, w2b_d[c])
                    for h in range(NH_T):
                        nc.tensor.matmul(pb, w2t[:, h * 128:(h + 1) * 128], hid[:, h, :],
                                         start=(h == 0), stop=(h == NH_T - 1))
                else:
                    w2t = sb.tile([128, NH_T, 128], FP8, tag="w2f", bufs=2, name=f"w2t{blk}_{c}")
                    nc.scalar.dma_start(w2t, w2f_d[c])
                    for h in range(0, NH_T, 2):
                        nc.tensor.matmul(pb, w2t[:, h:h + 2, :], hid[:, h:h + 2, :],
                                         start=(h == 0), stop=(h == NH_T - 2),
                                         perf_mode=DR)
                ot = sb.tile([128, tn], F32, tag="out", bufs=3, name=f"o{blk}_{c}", padded_shape=[128, NB])
                if has_beta:
                    nc.vector.tensor_mul(ot, pb, vcf)
                elif cls == "BB":
                    nc.vector.tensor_copy(ot, pb)
                else:
                    nc.vector.tensor_scalar_mul(ot, pb, DESC)
                nc.sync.dma_start(ygt_d[c * 128:(c + 1) * 128, tsl], ot)

        # Software pipeline: stats/normalize of blk+1 are emitted so the PE
        # runs them inside blk's mm1/mm2 stream with no gaps.
        vs0, vb0, vcf, xs0 = stats_phase(0)
        xn = normalize_phase(0, vs0, vb0, xs0)
        nxt = {}
        for blk in range(nblk):
            def mid_hook(b=blk):
                nxt.update(zip(("vs", "vb", "vcf", "xs"), stats_phase(b + 1)))
            hid = mm1_phase(blk, xn, mid_hook if blk + 1 < nblk else None)
            if blk + 1 < nblk:
                xn = normalize_phase(blk + 1, nxt["vs"], nxt["vb"], nxt["xs"])
            mm2_phase(blk, hid, vcf)
            if blk + 1 < nblk:
                vcf = nxt["vcf"]

    nc.compile()
    return nc


_KERNEL_CACHE = {}


def _get_kernel(NT: int, has_beta: bool):
    key = (NT, has_beta)
    if key not in _KERNEL_CACHE:
        _KERNEL_CACHE[key] = _build_kernel(NT, has_beta)
    return _KERNEL_CACHE[key]


def kernel(x, weights, gamma, beta, W1, W2, winners):
    x = np.asarray(x, dtype=np.float32)
    weights = np.asarray(weights, dtype=np.float32)
    gamma = np.asarray(gamma, dtype=np.float32)
    beta = np.asarray(beta, dtype=np.float32)
    W1 = np.asarray(W1, dtype=np.float32)
    W2 = np.asarray(W2, dtype=np.float32)
    winners = np.asarray(winners)

    B, T, C_ = x.shape
    E = W1.shape[0]
    assert C_ == C and E == N_CORES and W1.shape[2] == H

    x_flat = x.reshape(-1, C)
    win = winners.reshape(-1, 2)
    wts = weights.reshape(-1, 2)

    has_beta = bool(np.any(beta != 0.0))

    # ---- host-side routing (sharding prep) ----
    idxs, coefs = [], []
    for e in range(E):
        m = win == e
        tok = np.nonzero(m.any(axis=1))[0]
        cf = (wts * m).sum(axis=1)[tok]
        order = np.argsort(cf, kind="stable")
        idxs.append(tok[order])
        coefs.append(cf[order].astype(np.float32))
    NT = int(np.ceil(max(len(t) for t in idxs) / 8) * 8)

    in_maps = []
    for e in range(E):
        tok, cf = idxs[e], coefs[e]
        n = len(tok)
        pad = NT - n
        # pad at the FRONT: padding lands in the cheap fp8 class
        xg = np.zeros((NT, C), np.float32)
        xg[pad:] = x_flat[tok]
        cg = np.zeros((1, NT), np.float32)
        # fold sqrt(coef) into the LN scale (relu^2 is 2-homogeneous
        # and W2 linear, so scaling xn by sqrt(c) scales the output by c).
        cg[0, pad:] = cf if has_beta else np.sqrt(cf)
        w1g = W1[e] * gamma[:, None]
        w1sw = w1g.reshape(NC_T, 128, NH_T, 128).transpose(2, 1, 0, 3)
        w2sw = W2[e].reshape(NH_T, 128, NC_T, 128).transpose(2, 1, 0, 3)
        m = {
            "xgt": np.ascontiguousarray(xg.T.astype(NP_BF16)),
            "w1b": np.ascontiguousarray(w1sw.astype(NP_BF16)).reshape(NH_T, 128, C),
            "w2b": np.ascontiguousarray(w2sw.astype(NP_BF16)).reshape(NC_T, 128, H),
            "cg": cg,
        }
        if not has_beta:
            m["w1f"] = np.ascontiguousarray((w1sw * S_1).astype(NP_FP8))
            m["w2f"] = np.ascontiguousarray((w2sw * S_2).astype(NP_FP8))
        if has_beta:
            b1 = (beta @ W1[e]).astype(np.float32)          # [H]
            m["bias1"] = np.ascontiguousarray(b1.reshape(NH_T, 128).T)
        in_maps.append(m)

    nc = _get_kernel(NT, has_beta)
    # drop inputs the compiled program doesn't declare
    declared = {a.memorylocations[0].name
                for a in nc.m.functions[0].allocations
                if isinstance(a, mybir.MemoryLocationSet) and a.kind == "ExternalInput"}
    in_maps = [{k: v for k, v in im.items() if k in declared} for im in in_maps]
    res = run_bass_kernel_spmd(nc, in_maps, list(range(N_CORES)))

    # ---- host-side unshard: scatter-add partial expert outputs ----
    out = x_flat.copy()
    for e in range(E):
        yg = res.results[e]["ygt"]                          # [C, NT]
        n = len(idxs[e])
        pad = NT - n
        out[idxs[e]] += yg.T[pad:]
    return out.reshape(B, T, C).astype(np.float32)


# revision 5
# speedup vs baseline: 1.1807x; 1.1178x over previous
"""Trainium2 Bass kernel for CaMoE (LN + top-2 MoE with relu^2 FFN).

Strategy: expert-parallel over 8 NeuronCores with coef-routed mixed
precision. Core e receives the tokens routed to expert e (gather
indices computed host-side as part of sharding), sorted by combine
coefficient ascending. The first NF8 tokens (lowest coef) run both
matmuls in fp8-e4m3 DoubleRow (2x PE throughput), the next NBF run
mm1 in bf16 / mm2 in fp8 DoubleRow, the rest run fully in bf16. The
combine coefficient bounds each pair's contribution to the output, so
quantization error from the fp8 classes stays coef-proportional;
measured absmax/scale ~1.5e-2 vs the 2e-2 gate.

On device: LayerNorm stats via ones-matmul in replicated-lane form,
xn = (x - mu) * rstd * sqrt(coef) (relu^2 is 2-homogeneous and W2
linear, so scaling xn by sqrt(c) scales the output by c), hidden =
(relu(z)*sqrt(k))^2 with the class scale k folded into the Scalar
engine's relu, y = hidden @ W2, descaled and written back bf16
feature-major. Host scatter-adds the 8 partial outputs into x (the
residual) - pure unsharding, no collectives needed.

Self-contained: hardcodes shapes B=4, T=2048, C=1024, E=8, H=4096.
"""

import os
import sys

for _p in ("/opt/trn_rl_repo", "/root/.axon_site/_ro/trn_rl_repo"):
    if os.path.isdir(_p) and _p not in sys.path:
        sys.path.insert(0, _p)

from contextlib import ExitStack

import ml_dtypes
import numpy as np

import concourse.bass as bass
import concourse.tile as tile
from concourse import bacc, mybir
from concourse.bass_utils import run_bass_kernel_spmd

N_CORES = 8
C = 1024
H = 4096
NB = 512          # token block (matmul moving free dim)
NC_T = C // 128   # 8 c-tiles
NH_T = H // 128   # 32 h-tiles
EPS = 1e-5

# mixed-precision class sizes (tokens, sorted by coef ascending)
NF8 = 768         # both matmuls fp8 DoubleRow
NBF = 512         # mm1 bf16, mm2 fp8 DoubleRow
# fp8 scale factors
S_X = 16.0        # xn pre-scale (fp8 class FF)
S_1 = 128.0       # W1 pre-scale (fp8)
S_H = 4.0         # hidden pre-scale (fp8)
S_2 = 256.0       # W2 pre-scale (fp8)

F32 = mybir.dt.float32
BF16 = mybir.dt.bfloat16
FP8 = mybir.dt.float8e4
DR = mybir.MatmulPerfMode.DoubleRow
AF = mybir.ActivationFunctionType
OP = mybir.AluOpType
NP_FP8 = mybir.dt.np(FP8)
NP_BF16 = mybir.dt.np(BF16)


def _block_list(NT, nf8, nbf):
    """[(t0, tn, cls)] covering [0, NT). FF blocks ordered small-first."""
    blocks = []

    def span(lo, hi, cls, warmup=False):
        sizes = []
        rem = hi - lo
        if warmup:
            # two small leading blocks shorten the cold-start chain
            while rem > NB and len(sizes) < 2:
                sizes.append(128)
                rem -= 128
        while rem > 0:
            tn = min(NB, rem)
            sizes.append(tn)
            rem -= tn
        sizes.sort()
        t = lo
        for sz in sizes:
            blocks.append((t, sz, cls))
            t += sz

    b0 = min(nf8, NT)
    b1 = min(nf8 + nbf, NT)
    span(0, b0, "FF", warmup=True)
    span(b0, b1, "BF")
    span(b1, NT, "BB")
    return blocks


def _build_kernel(NT: int, has_beta: bool):
    """Build the per-core SPMD program for NT padded tokens."""
    nf8, nbf = (0, 0) if has_beta else (NF8, NBF)
    blocks = _block_list(NT, nf8, nbf)
    nblk = len(blocks)
    any_ff = any(b[2] == "FF" for b in blocks)
    any_f8mm2 = any(b[2] in ("FF", "BF") for b in blocks)
    any_bf16mm1 = any(b[2] in ("BF", "BB") for b in blocks)
    any_bb = any(b[2] == "BB" for b in blocks)

    nc = bacc.Bacc("TRN2", target_bir_lowering=False, debug=False, num_devices=1)

    # x and y are stored [128, NC_T, NT] (partition-major) so one DMA
    # moves a whole block; weights are pre-swizzled into per-tile lhsT
    # layout, w1 packed in h-tile pairs so one DMA feeds two h-tiles.
    xgt_d = nc.dram_tensor("xgt", [128, NC_T, NT], BF16, kind="ExternalInput").ap()
    if any_bf16mm1:
        w1b_d = nc.dram_tensor("w1b", [NH_T // 2, 128, 2 * C], BF16,
                               kind="ExternalInput").ap()
    if any_ff:
        w1f_d = nc.dram_tensor("w1f", [NH_T // 2, 128, 2 * NC_T, 128], FP8,
                               kind="ExternalInput").ap()
    if any_bb:
        w2b_d = nc.dram_tensor("w2b", [NC_T, 128, H], BF16, kind="ExternalInput").ap()
    if any_f8mm2:
        w2f_d = nc.dram_tensor("w2f", [NC_T, 128, NH_T, 128], FP8,
                               kind="ExternalInput").ap()
    cg_d = nc.dram_tensor("cg", [1, NT], F32, kind="ExternalInput").ap()
    if has_beta:
        bias1_d = nc.dram_tensor("bias1", [128, NH_T], F32, kind="ExternalInput").ap()
    ygt_d = nc.dram_tensor("ygt", [128, NC_T, NT], BF16, kind="ExternalOutput").ap()

    # relu scale sqrt(k) per class; hid = (relu(z * sqrt(k)))^2 = k*relu(z)^2
    RS = {"FF": float(np.sqrt(S_H)) / (S_X * S_1), "BF": float(np.sqrt(S_H)),
          "BB": 1.0}
    DESC = 1.0 / (S_H * S_2)

    with tile.TileContext(nc) as tc, ExitStack() as ctx:
        sb = ctx.enter_context(tc.tile_pool(name="sb", bufs=1))
        ps = ctx.enter_context(tc.tile_pool(name="ps", bufs=1, space="PSUM"))

        # ---- constants ----
        ones_k = sb.tile([128, 128], BF16, tag="ones_k", bufs=1)
        nc.vector.memset(ones_k, 1.0)
        eps_t = sb.tile([128, 1], F32, tag="eps", bufs=1)
        nc.vector.memset(eps_t, EPS)
        if has_beta:
            b1sb = sb.tile([128, NH_T], F32, tag="b1", bufs=1)
            nc.sync.dma_start(b1sb, bias1_d)

        def stats_load(blk, split_first=False):
            """DMA x for block blk + per-c-tile squares (Scalar)."""
            t0, tn, cls = blocks[blk]
            tsl = bass.ds(t0, tn)
            xs3 = sb.tile([128, NC_T, tn], BF16, tag="xs", bufs=2,
                          name=f"xa{blk}", padded_shape=[128, NC_T, NB])
            if split_first:
                # block 0: two half DMAs so squares start earlier
                nc.sync.dma_start(xs3[:, 0:4, :], xgt_d[:, 0:4, tsl])
                nc.sync.dma_start(xs3[:, 4:8, :], xgt_d[:, 4:8, tsl])
            else:
                nc.sync.dma_start(xs3, xgt_d[:, :, tsl])
            xsqs = []
            for c in range(NC_T):
                xsq = sb.tile([128, tn], BF16, tag="xsq", bufs=3,
                              name=f"xsq{blk}_{c}", padded_shape=[128, NB])
                nc.scalar.activation(xsq, xs3[:, c, :], AF.Square)
                xsqs.append(xsq)
            vcg = sb.tile([128, tn], F32, tag="bc", bufs=5, name=f"vcg{blk}", padded_shape=[128, NB])
            nc.sync.dma_start(vcg, cg_d[0:1, tsl].to_broadcast([128, tn]))
            return xs3, xsqs, vcg

        def stats_calc(blk, loaded):
            """LN stats for block blk, replicated-lane form."""
            t0, tn, cls = blocks[blk]
            xs3, xsqs, vcg = loaded
            sum_ps = ps.tile([128, tn], F32, tag="stat", bufs=3, name=f"sum{blk}")
            sq_ps = ps.tile([128, tn], F32, tag="stat", bufs=3, name=f"sq{blk}")
            for c in range(NC_T):
                nc.tensor.matmul(sum_ps, ones_k, xs3[:, c, :],
                                 start=(c == 0), stop=(c == NC_T - 1))
                nc.tensor.matmul(sq_ps, ones_k, xsqs[c],
                                 start=(c == 0), stop=(c == NC_T - 1))
            vmu = sb.tile([128, tn], F32, tag="vec", bufs=3, name=f"vmu{blk}", padded_shape=[128, NB])
            nc.vector.tensor_scalar_mul(vmu, sum_ps, 1.0 / C)
            # var = sq/C - mu^2
            vvar = sb.tile([128, tn], F32, tag="vec", bufs=3, name=f"vvar{blk}", padded_shape=[128, NB])
            nc.vector.scalar_tensor_tensor(vvar, vmu, -1.0, vmu, OP.mult, OP.mult)
            nc.vector.scalar_tensor_tensor(vvar, sq_ps, 1.0 / C, vvar, OP.mult, OP.add)
            vstd = sb.tile([128, tn], F32, tag="vec", bufs=3, name=f"vstd{blk}", padded_shape=[128, NB])
            nc.scalar.activation(vstd, vvar, AF.Sqrt, bias=eps_t)
            vrstd = sb.tile([128, tn], F32, tag="vec", bufs=3, name=f"vrstd{blk}", padded_shape=[128, NB])
            nc.vector.reciprocal_approx_fast(out=vrstd, in_=vstd)
            if has_beta:
                vs = vrstd          # coef applied on the output instead
            else:
                vs = sb.tile([128, tn], F32, tag="bc", bufs=5, name=f"vs{blk}", padded_shape=[128, NB])
                sxf = S_X if blocks[blk][2] == "FF" else 1.0
                nc.vector.scalar_tensor_tensor(vs, vrstd, sxf, vcg, OP.mult, OP.mult)
            vb = sb.tile([128, tn], F32, tag="bc", bufs=5, name=f"vb{blk}", padded_shape=[128, NB])
            nc.vector.scalar_tensor_tensor(vb, vmu, -1.0, vs, OP.mult, OP.mult)
            return vs, vb, vcg, xs3

        def normalize_phase(blk, vs, vb, xs3):
            t0, tn, cls = blocks[blk]
            if cls == "FF":
                xn = sb.tile([128, NC_T, tn], FP8, tag="xnf", bufs=1,
                             name=f"xn{blk}", padded_shape=[128, NC_T, NB])
            else:
                xn = sb.tile([128, NC_T, tn], BF16, tag="xnb", bufs=1,
                             name=f"xn{blk}", padded_shape=[128, NC_T, NB])
            for c in range(NC_T):
                tmp = sb.tile([128, tn], F32, tag="tmp", bufs=2, name=f"tp{blk}_{c}", padded_shape=[128, NB])
                nc.vector.tensor_mul(tmp, xs3[:, c, :], vs)
                nc.vector.tensor_add(xn[:, c, :], tmp, vb)
            return xn

        def pack2(sz):
            """largest power of 2 <= 512//sz (PSUM-bank packing factor)"""
            g = 1
            while 2 * g * sz <= NB and 2 * g <= 8:
                g *= 2
            return g

        def w1_load(blk, hh):
            cls = blocks[blk][2]
            if cls == "FF":
                w1t = sb.tile([128, 2 * NC_T, 128], FP8, tag="w1f",
                              bufs=4, name=f"w1t{blk}_{hh}")
                nc.sync.dma_start(w1t, w1f_d[hh])
            else:
                w1t = sb.tile([128, 2 * C], BF16, tag="w1s",
                              bufs=4, name=f"w1t{blk}_{hh}")
                nc.sync.dma_start(w1t, w1b_d[hh])
            return w1t

        def w2_load(blk, c):
            cls = blocks[blk][2]
            if cls == "BB":
                w2t = sb.tile([128, H], BF16, tag="w2s", bufs=4,
                              name=f"w2t{blk}_{c}")
                nc.sync.dma_start(w2t, w2b_d[c])
            else:
                w2t = sb.tile([128, NH_T, 128], FP8, tag="w2f", bufs=4,
                              name=f"w2t{blk}_{c}")
                nc.sync.dma_start(w2t, w2f_d[c])
            return w2t

        def mm1_phase(blk, xn, w1pre, hook_load=None, hook_calc=None):
            t0, tn, cls = blocks[blk]
            G = pack2(tn)               # h-tiles packed per PSUM bank
            if cls == "BB":
                hid = sb.tile([128, NH_T, tn], BF16, tag="hidb", bufs=1,
                              name=f"hid{blk}", padded_shape=[128, NH_T, NB])
            else:
                hid = sb.tile([128, NH_T, tn], FP8, tag="hidf", bufs=1,
                              name=f"hid{blk}", padded_shape=[128, NH_T, NB])
            w1tiles = list(w1pre)
            w2pre = []
            pa = None
            for h in range(NH_T):
                if h == 4 and hook_load is not None:
                    hook_load()
                if h == 26 and hook_calc is not None:
                    hook_calc()
                if h == 26:
                    w2pre = [w2_load(blk, 0), w2_load(blk, 1)]
                if h % 2 == 0:
                    # keep ~2 weight-pair DMAs in flight ahead of use
                    while len(w1tiles) <= min(h // 2 + 2, NH_T // 2 - 1):
                        w1tiles.append(w1_load(blk, len(w1tiles)))
                    w1t, j = w1tiles[h // 2], 0
                else:
                    j = 1
                if h % G == 0:
                    pa = ps.tile([128, G, tn], F32, tag="mm", bufs=4,
                                 name=f"pa{blk}_{h}",
                                 padded_shape=[128, G, NB // G])
                g = h % G
                if cls == "FF":
                    for c in range(0, NC_T, 2):
                        nc.tensor.matmul(pa[:, g, :],
                                         w1t[:, j * NC_T + c:j * NC_T + c + 2, :],
                                         xn[:, c:c + 2, :],
                                         start=(c == 0), stop=(c == NC_T - 2),
                                         perf_mode=DR)
                else:
                    for c in range(NC_T):
                        nc.tensor.matmul(pa[:, g, :],
                                         w1t[:, j * C + c * 128:j * C + (c + 1) * 128],
                                         xn[:, c, :],
                                         start=(c == 0), stop=(c == NC_T - 1))
                if g == G - 1:
                    h0 = h - G + 1
                    rt = sb.tile([128, G, tn], BF16, tag="rt", bufs=3,
                                 name=f"r{blk}_{h0}", padded_shape=[128, G, NB // G])
                    if has_beta:
                        for gg in range(G):
                            nc.vector.tensor_scalar_add(
                                pa[:, gg, :], pa[:, gg, :],
                                b1sb[:, h0 + gg:h0 + gg + 1])
                    # hid = (relu(z*sqrt(k)))^2 = k*relu(z)^2
                    nc.scalar.activation(rt, pa, AF.Relu, scale=RS[cls])
                    nc.vector.tensor_mul(hid[:, h0:h0 + G, :], rt, rt)
            return hid, w2pre

        def mm2_phase(blk, hid, vcf, w2pre, prefetch_next):
            t0, tn, cls = blocks[blk]
            P = pack2(tn)               # c-tiles packed per PSUM bank
            tsl = bass.ds(t0, tn)
            w1pre_next = []
            if prefetch_next:
                w1pre_next = [w1_load(blk + 1, 0), w1_load(blk + 1, 1)]
            w2tiles = list(w2pre)
            pb = None
            for c in range(NC_T):
                while len(w2tiles) <= min(c + 2, NC_T - 1):
                    w2tiles.append(w2_load(blk, len(w2tiles)))
                w2t = w2tiles[c]
                if c % P == 0:
                    pb = ps.tile([128, P, tn], F32, tag="mm", bufs=4,
                                 name=f"pb{blk}_{c}",
                                 padded_shape=[128, P, NB // P])
                p = c % P
                if cls == "BB":
                    for h in range(NH_T):
                        nc.tensor.matmul(pb[:, p, :], w2t[:, h * 128:(h + 1) * 128],
                                         hid[:, h, :],
                                         start=(h == 0), stop=(h == NH_T - 1))
                else:
                    for h in range(0, NH_T, 2):
                        nc.tensor.matmul(pb[:, p, :], w2t[:, h:h + 2, :],
                                         hid[:, h:h + 2, :],
                                         start=(h == 0), stop=(h == NH_T - 2),
                                         perf_mode=DR)
                if p == P - 1:
                    c0 = c - P + 1
                    ot = sb.tile([128, P, tn], BF16, tag="out", bufs=3,
                                 name=f"o{blk}_{c0}", padded_shape=[128, P, NB // P])
                    if has_beta:
                        for pp in range(P):
                            nc.vector.tensor_mul(ot[:, pp, :], pb[:, pp, :], vcf)
                    elif cls == "BB":
                        nc.vector.tensor_copy(ot, pb)
                    else:
                        nc.vector.tensor_scalar_mul(ot, pb, DESC)
                    nc.sync.dma_start(ygt_d[:, c0:c0 + P, tsl], ot)
            return w1pre_next

        # Software pipeline: stats of blk+1 load early / compute late
        # inside blk's mm1 so the PE never waits at a block boundary.
        ld0 = stats_load(0, split_first=True)
        w1pre = [w1_load(0, 0), w1_load(0, 1)]
        vs0, vb0, vcf, xs0 = stats_calc(0, ld0)
        xn = normalize_phase(0, vs0, vb0, xs0)
        nxt = {}
        for blk in range(nblk):
            def hook_load(b=blk):
                nxt["ld"] = stats_load(b + 1)

            def hook_calc(b=blk):
                nxt.update(zip(("vs", "vb", "vcf", "xs"),
                               stats_calc(b + 1, nxt["ld"])))
            last = blk + 1 >= nblk
            hid, w2pre = mm1_phase(blk, xn, w1pre,
                                   None if last else hook_load,
                                   None if last else hook_calc)
            if not last:
                xn = normalize_phase(blk + 1, nxt["vs"], nxt["vb"], nxt["xs"])
            w1pre = mm2_phase(blk, hid, vcf, w2pre, not last)
            if not last:
                vcf = nxt["vcf"]

    nc.compile()
    return nc


_KERNEL_CACHE = {}


def _get_kernel(NT: int, has_beta: bool):
    key = (NT, has_beta)
    if key not in _KERNEL_CACHE:
        _KERNEL_CACHE[key] = _build_kernel(NT, has_beta)
    return _KERNEL_CACHE[key]


def kernel(x, weights, gamma, beta, W1, W2, winners):
    x = np.asarray(x, dtype=np.float32)
    weights = np.asarray(weights, dtype=np.float32)
    gamma = np.asarray(gamma, dtype=np.float32)
    beta = np.asarray(beta, dtype=np.float32)
    W1 = np.asarray(W1, dtype=np.float32)
    W2 = np.asarray(W2, dtype=np.float32)
    winners = np.asarray(winners)

    B, T, C_ = x.shape
    E = W1.shape[0]
    assert C_ == C and E == N_CORES and W1.shape[2] == H

    x_flat = x.reshape(-1, C)
    win = winners.reshape(-1, 2)
    wts = weights.reshape(-1, 2)

    has_beta = bool(np.any(beta != 0.0))

    # ---- host-side routing (sharding prep) ----
    idxs, coefs = [], []
    for e in range(E):
        m = win == e
        tok = np.nonzero(m.any(axis=1))[0]
        cf = (wts * m).sum(axis=1)[tok]
        order = np.argsort(cf, kind="stable")
        idxs.append(tok[order])
        coefs.append(cf[order].astype(np.float32))
    NT = int(np.ceil(max(len(t) for t in idxs) / 8) * 8)

    in_maps = []
    for e in range(E):
        tok, cf = idxs[e], coefs[e]
        n = len(tok)
        pad = NT - n
        # pad at the FRONT: padding lands in the cheap fp8 class
        xg = np.zeros((NT, C), np.float32)
        xg[pad:] = x_flat[tok]
        cg = np.zeros((1, NT), np.float32)
        # fold sqrt(coef) into the LN scale (relu^2 is 2-homogeneous
        # and W2 linear, so scaling xn by sqrt(c) scales the output by c).
        cg[0, pad:] = cf if has_beta else np.sqrt(cf)
        # x stored partition-major: xgt[p, c, t] = x[tok[t], c*128+p]
        xg3 = np.ascontiguousarray(
            xg.T.reshape(NC_T, 128, NT).transpose(1, 0, 2).astype(NP_BF16))
        w1g = W1[e] * gamma[:, None]
        w1sw = w1g.reshape(NC_T, 128, NH_T, 128).transpose(2, 1, 0, 3)
        # pack h-tile pairs: [NH_T//2, 128, 2, NC_T, 128]
        w1pair = w1sw.reshape(NH_T // 2, 2, 128, NC_T, 128).transpose(0, 2, 1, 3, 4)
        w2sw = W2[e].reshape(NH_T, 128, NC_T, 128).transpose(2, 1, 0, 3)
        m = {
            "xgt": xg3,
            "w1b": np.ascontiguousarray(w1pair.astype(NP_BF16)).reshape(
                NH_T // 2, 128, 2 * C),
            "w2b": np.ascontiguousarray(w2sw.astype(NP_BF16)).reshape(NC_T, 128, H),
            "cg": cg,
        }
        if not has_beta:
            m["w1f"] = np.ascontiguousarray((w1pair * S_1).astype(NP_FP8)).reshape(
                NH_T // 2, 128, 2 * NC_T, 128)
            m["w2f"] = np.ascontiguousarray((w2sw * S_2).astype(NP_FP8))
        if has_beta:
            b1 = (beta @ W1[e]).astype(np.float32)          # [H]
            m["bias1"] = np.ascontiguousarray(b1.reshape(NH_T, 128).T)
        in_maps.append(m)

    nc = _get_kernel(NT, has_beta)
    # drop inputs the compiled program doesn't declare
    declared = {a.memorylocations[0].name
                for a in nc.m.functions[0].allocations
                if isinstance(a, mybir.MemoryLocationSet) and a.kind == "ExternalInput"}
    in_maps = [{k: v for k, v in im.items() if k in declared} for im in in_maps]
    res = run_bass_kernel_spmd(nc, in_maps, list(range(N_CORES)))

    # ---- host-side unshard: scatter-add partial expert outputs ----
    out = x_flat.copy()
    for e in range(E):
        yg = res.results[e]["ygt"]                          # [128, NC_T, NT]
        n = len(idxs[e])
        pad = NT - n
        yt = yg.transpose(2, 1, 0).reshape(NT, C).astype(np.float32)
        out[idxs[e]] += yt[pad:]
    return out.reshape(B, T, C).astype(np.float32)
